# revision 39
# baseline (speedup 1.0000x reference)
"""BitNet SwiGLU MLP kernel for Trainium2, tensor-parallel over 8 NeuronCores.

Sharding (Megatron-style TP over the intermediate dim F):
- Each core holds a 1/8 column-shard of Wg/Wu (fed transposed: [D, FS]) and
  the matching shard of Wd (fed as Wd[:, shard].T = [FS, D]). x is replicated,
  fed both natural-sliced (per-token quant stats, sharded over tokens) and
  fully transposed [D, T] (matmul operand layout).
- bit_linear runs as an exact integer matmul in bf16: quantized activations
  are ints in [-128,127] and ternary weights in {-1,0,1} (both exact in bf16,
  accumulated exactly in fp32 PSUM), dequantized on the output by per-token /
  global scales. clip(round(.)) for activations is exact RNE via the fp32
  magic-number trick (the clip provably never binds since |x*scale| <= 127).
- Down-proj + mean-pool is collapsed algebraically:
  mean_{h,d}(hq @ Wdq.T) = 1/(H*D) * sum_f hq[t,f] * S[f],  S = colsum(Wdq)
  so only a per-token weighted row-reduction against S remains.
- h is kept resident in SBUF as fp16 between the main loop and the
  per-token requantization pass (which needs globally AllGathered RMS
  stats); no DRAM roundtrip.
- All streaming elementwise work runs on Vector/Scalar (never GpSimd, which
  is ~7x slower for multi-op tensor_scalar); GpSimd only triggers
  collectives, does the small cross-partition reductions/broadcasts, and the
  weight |.| column statistics at the head.
- Pooled partials are AllReduced; every core runs the tiny classifier.
"""
import numpy as np

MAGIC = 12582912.0  # 1.5 * 2^23, fp32 RNE magic
EPS = 1e-6
QEPS = 1e-5


def build(B=8, C=3, H=128, D=2048, F=8192, NCLS=1000, NCORES=8,
          ln_is_ones=True, mock_collectives=False,
          NDUM1=270, NDUM2=30, H_BUFS=10, XQ_BUFS=20,
          SAFE_CLIP=True, SAFE_BCAST=True, SAFE_GATHER=True, SAFE_TTR=True):
    """Build + compile the SPMD Bass program. Returns (nc, meta)."""
    import concourse.bacc as bacc
    import concourse.tile as tile
    from concourse import mybir
    from concourse import bass_isa

    f32 = mybir.dt.float32
    bf16 = mybir.dt.bfloat16
    fp16 = mybir.dt.float16
    AX = mybir.AxisListType
    OP = mybir.AluOpType
    AF = mybir.ActivationFunctionType
    RO = bass_isa.ReduceOp
    RG = [list(range(NCORES))]

    assert H == 128
    T = B * C * H
    TT = T // 128               # token tiles (== B*C) = 24
    TS = T // NCORES            # tokens per core for x stats
    TST = TS // 128
    FS = F // NCORES            # f-shard width = 1024
    DT = D // 128               # contraction tiles = 16
    NF = 512
    FH = FS // NF               # = 2
    WB = 2                      # d-tiles per weight/x DMA slab

    # token-tile groups (last ones smaller to tighten the tail)
    GROUPS = [(0, 4), (4, 4), (8, 4), (12, 4), (16, 4), (20, 2), (22, 2)]
    assert sum(g[1] for g in GROUPS) == TT
    # stat segments, aligned to groups; post-stats for segment s runs two
    # groups after the segment's last tile so its AllGather can complete.
    SEGB = [0, 4, 8, 12, 16, 20, 22, 24]
    NSEG = len(SEGB) - 1
    def group_of(t):
        for gi, (t0, gsz) in enumerate(GROUPS):
            if t0 <= t < t0 + gsz:
                return gi
        raise AssertionError
    SEG_OF = {}
    for s in range(NSEG):
        for t in range(SEGB[s], SEGB[s + 1]):
            SEG_OF[t] = s
    POST_AT = {}          # group index -> [segments to post before this group]
    POST_TAIL = []
    for s in range(NSEG):
        gdone = group_of(SEGB[s + 1] - 1)
        gpost = gdone + (2 if gdone <= 4 else 1)
        if gpost < len(GROUPS):
            POST_AT.setdefault(gpost, []).append(s)
        else:
            POST_TAIL.append(s)

    nc = bacc.Bacc("TRN2", target_bir_lowering=False, debug=False,
                   num_devices=1 if mock_collectives else NCORES)

    def collective(kind, op, in_ap, out_ap):
        if NCORES == 1 or mock_collectives:
            n = out_ap.size() // in_ap.size()
            flat = out_ap.rearrange("a b -> (a b)")
            for r in range(n):
                nc.sync.dma_start(
                    flat[r * in_ap.size():(r + 1) * in_ap.size()], in_ap)
        else:
            nc.gpsimd.collective_compute(kind, op, replica_groups=RG,
                                         ins=[in_ap.opt()], outs=[out_ap.opt()])

    xs_t = nc.dram_tensor("xs", [TS, D], f32, kind="ExternalInput")
    xT_t = nc.dram_tensor("xT", [D, T], f32, kind="ExternalInput")
    wgT_t = nc.dram_tensor("wgT", [D, FS], f32, kind="ExternalInput")
    wuT_t = nc.dram_tensor("wuT", [D, FS], f32, kind="ExternalInput")
    wdT_t = nc.dram_tensor("wdT", [FS, D], f32, kind="ExternalInput")
    lnw_t = nc.dram_tensor("lnw", [1, FS], f32, kind="ExternalInput")
    clsWT_t = nc.dram_tensor("clsWT", [C, NCLS], f32, kind="ExternalInput")
    clsb_t = nc.dram_tensor("clsb", [1, NCLS], f32, kind="ExternalInput")
    out_t = nc.dram_tensor("out", [B, NCLS], f32, kind="ExternalOutput")

    def r128(ap):
        # [1, n*128] dram view -> [128, n] (partition = fast axis)
        return ap.rearrange("o (i p) -> (o p) i", p=128)

    with tile.TileContext(nc) as tc:
        import contextlib
        with contextlib.ExitStack() as st:
            dram = st.enter_context(tc.tile_pool(name="dram", bufs=1, space="DRAM"))
            sbC = st.enter_context(tc.tile_pool(name="sbC", bufs=1))
            sbS8 = st.enter_context(tc.tile_pool(name="sbS8", bufs=2))
            sbT1 = st.enter_context(tc.tile_pool(name="sbT1", bufs=2))
            sbS2 = st.enter_context(tc.tile_pool(name="sbS2", bufs=3))
            sbUG = st.enter_context(tc.tile_pool(name="sbUG", bufs=2))
            sbSG = st.enter_context(tc.tile_pool(name="sbSG", bufs=2))
            sbCol = st.enter_context(tc.tile_pool(name="sbCol", bufs=4))
            sbH = st.enter_context(tc.tile_pool(name="sbH", bufs=H_BUFS))
            sbXQ = st.enter_context(tc.tile_pool(name="sbXQ", bufs=XQ_BUFS))
            sbWQ = st.enter_context(tc.tile_pool(name="sbWQ", bufs=2 * DT))

            sc_in = dram.tile([1, TS], f32)
            sc_out = dram.tile([1, T], f32)
            c1_in = dram.tile([1, 8], f32)
            c1_out = dram.tile([1, 8], f32)
            c2_in = dram.tile([1, 8], f32)
            c2_out = dram.tile([1, 8], f32)
            srow_dram = dram.tile([1, FS], f32)
            dum_dram = dram.tile([1, 8], f32)
            st_in = [dram.tile([256, SEGB[s + 1] - SEGB[s]], f32,
                               name=f"st_in{s}") for s in range(NSEG)]
            st_out = [dram.tile([256 * NCORES, SEGB[s + 1] - SEGB[s]], f32,
                                name=f"st_out{s}") for s in range(NSEG)]
            pl_in = dram.tile([1, TT], f32)
            pl_out = dram.tile([1, TT], f32)

            ones1 = sbC.tile([1, 128], f32)
            nc.vector.memset(ones1[:], 1.0)
            ones_col = sbC.tile([128, 1], f32)
            nc.vector.memset(ones_col[:], 1.0)
            negmagic = sbC.tile([128, 1], f32)
            nc.vector.memset(negmagic[:], -MAGIC)
            zeros_bf = sbC.tile([128, NF], bf16)
            nc.vector.memset(zeros_bf[:], 0.0)

            wacc = sbC.tile([128, 24], f32)
            sc_cols = sbC.tile([128, TST], f32)
            c1_sb = sbC.tile([1, 8], f32)
            c2_sb = sbC.tile([1, 8], f32)
            m_w_col = sbC.tile([128, 2], f32)
            s_w_col = sbC.tile([128, 2], f32)
            m_wd_col = sbC.tile([128, 1], f32)
            s_wd_col = sbC.tile([128, 1], f32)
            S_bcast = sbC.tile([128, T], f32)
            Sh_bcast = sbC.tile([128, FS], f32)
            DEQG = sbC.tile([128, TT], f32)
            DEQU = sbC.tile([128, TT], f32)
            ssq_cols = sbC.tile([128, TT], f32)
            am_cols = sbC.tile([128, TT], f32)
            Q_cols = sbC.tile([128, TT], f32)
            Ssh_cols = sbC.tile([128, FS // 128], f32)
            if not ln_is_ones:
                Ln_bcast = sbC.tile([128, FS], f32)

            ht_tiles = [None] * TT

            def bcast_row(out_cols, in_row, n):
                # out_cols [128, n] <- broadcast of in_row [1, n]
                if SAFE_BCAST:
                    pad = sbCol.tile([128, max(n, 1)], f32, tag="bc",
                                     bufs=2, name="bcpad")
                    nc.vector.memset(pad[:, 0:n], 0.0)
                    nc.vector.tensor_copy(pad[0:1, 0:n], in_row[0:1, 0:n])
                    nc.gpsimd.partition_all_reduce(out_cols[:, 0:n],
                                                   pad[:, 0:n], channels=128,
                                                   reduce_op=RO.add)
                else:
                    nc.gpsimd.partition_broadcast(out_cols[:, 0:n],
                                                  in_row[0:1, 0:n],
                                                  channels=128)

            def clip_step(t1):
                # clamp t1 (rounded magic form) to [MAGIC-1, MAGIC+1]
                if SAFE_CLIP:
                    t2 = sbT1.tile([128, FS], f32, tag="T2", bufs=2)
                    nc.vector.tensor_scalar(out=t2[:], in0=t1[:],
                                            scalar1=MAGIC - 1.0,
                                            scalar2=MAGIC + 1.0,
                                            op0=OP.max, op1=OP.min)
                    return t2
                nc.vector.tensor_scalar(out=t1[:], in0=t1[:],
                                        scalar1=MAGIC - 1.0,
                                        scalar2=MAGIC + 1.0,
                                        op0=OP.max, op1=OP.min)
                return t1

            def mult_rowsum(in0, in1, accum, init):
                # accum [128,1] = init + sum_cols(in0 * in1)
                if SAFE_TTR:
                    jk = sbUG.tile([128, FS], bf16, tag="jkf", bufs=2,
                                   name="jkf")
                    nc.vector.tensor_tensor(out=jk[:], in0=in0, in1=in1,
                                            op=OP.mult)
                    if init is None:
                        nc.vector.tensor_reduce(out=accum[:], in_=jk[:],
                                                axis=AX.X, op=OP.add)
                    else:
                        part = sbCol.tile([128, 1], f32, tag="mrs",
                                          name="mrs")
                        nc.vector.tensor_reduce(out=part[:], in_=jk[:],
                                                axis=AX.X, op=OP.add)
                        nc.vector.tensor_tensor(out=accum[:], in0=part[:],
                                                in1=init[:], op=OP.add)
                else:
                    jk = sbUG.tile([128, FS], bf16, tag="jk", bufs=2,
                                   name="jk")
                    nc.vector.tensor_tensor_reduce(
                        out=jk[:], in0=in0, in1=in1, scale=1.0,
                        scalar=(0.0 if init is None else init[:]),
                        op0=OP.mult, op1=OP.add, accum_out=accum[:])

            # ================= Head =================
            with tc.tile_pool(name="psH", bufs=1, space="PSUM") as psH:
                # dummy matmuls keep the PE HAM-warm until real MMs arrive
                if NDUM1 > 0:
                    dum = psH.tile([128, NF], f32, tag="dum")
                    for k in range(NDUM1):
                        nc.tensor.matmul(dum[:], zeros_bf[:, 0:128],
                                         zeros_bf[:], start=(k == 0),
                                         stop=(k == NDUM1 - 1))

                # x per-token clipped absmax -> sc_in -> AllGather
                for i in range(TST):
                    xt = sbS8.tile([128, D], f32, tag="S8")
                    nc.sync.dma_start(xt[:], xs_t.ap()[i * 128:(i + 1) * 128, :])
                    am = sbCol.tile([128, 1], f32, tag="am")
                    nc.vector.tensor_reduce(out=am[:], in_=xt[:], axis=AX.X,
                                            op=OP.max, apply_absolute_value=True)
                    nc.vector.tensor_scalar(out=sc_cols[:, i:i + 1], in0=am[:],
                                            scalar1=QEPS, scalar2=None,
                                            op0=OP.max)
                nc.sync.dma_start(r128(sc_in[:]), sc_cols[:])

                # Wg/Wu |.| sums (gpsimd reduces; vector+PE stay free)
                qeng = [nc.sync, nc.gpsimd, nc.sync, nc.gpsimd]
                for i in range(0, DT, WB):
                    for j, ten in enumerate((wgT_t, wuT_t)):
                        wt = sbS8.tile([128, WB * FS], f32, tag="S8")
                        qeng[(i // WB * 2 + j) % 4].dma_start(
                            wt[:],
                            ten.ap()[i * 128:(i + WB) * 128, :]
                            .rearrange("(b p) c -> p b c", p=128))
                        nc.vector.tensor_reduce(
                            out=wacc[:, j * 8 + i // WB:j * 8 + i // WB + 1],
                            in_=wt[:], axis=AX.X, op=OP.add,
                            apply_absolute_value=True)
                colg = sbCol.tile([128, 1], f32, tag="cg")
                nc.vector.tensor_reduce(out=colg[:], in_=wacc[:, 0:8],
                                        axis=AX.X, op=OP.add)
                colu = sbCol.tile([128, 1], f32, tag="cu")
                nc.vector.tensor_reduce(out=colu[:], in_=wacc[:, 8:16],
                                        axis=AX.X, op=OP.add)
                smg = psH.tile([1, 1], f32, tag="sm", bufs=3, name="smg")
                nc.tensor.matmul(smg[:], colg[:], ones_col[:],
                                 start=True, stop=True)
                smu = psH.tile([1, 1], f32, tag="sm", bufs=3, name="smu")
                nc.tensor.matmul(smu[:], colu[:], ones_col[:],
                                 start=True, stop=True)
                nc.vector.memset(c1_sb[:], 0.0)
                nc.vector.tensor_copy(c1_sb[:, 0:1], smg[:])
                nc.vector.tensor_copy(c1_sb[:, 1:2], smu[:])
                nc.sync.dma_start(c1_in[:], c1_sb[:])
                collective("AllReduce", OP.add, c1_in[:], c1_out[:])
                # sc AllGather triggered after c1 so the weight-scale chain
                # (which gates the whole matmul stream) clears CC first
                collective("AllGather", OP.bypass, sc_in[:], sc_out[:])

                # Wd |.| sums -> c2 AllReduce
                for i in range(FS // 128):
                    wt = sbS8.tile([128, D], f32, tag="S8")
                    qeng[i % 4].dma_start(wt[:],
                                          wdT_t.ap()[i * 128:(i + 1) * 128, :])
                    nc.vector.tensor_reduce(
                        out=wacc[:, 16 + i:17 + i], in_=wt[:], axis=AX.X,
                        op=OP.add, apply_absolute_value=True)
                cold = sbCol.tile([128, 1], f32, tag="cd")
                nc.vector.tensor_reduce(out=cold[:], in_=wacc[:, 16:24],
                                        axis=AX.X, op=OP.add)
                smd = psH.tile([1, 1], f32, tag="sm", bufs=3, name="smd")
                nc.tensor.matmul(smd[:], cold[:], ones_col[:],
                                 start=True, stop=True)
                nc.vector.memset(c2_sb[:], 0.0)
                nc.vector.tensor_copy(c2_sb[:, 0:1], smd[:])
                nc.sync.dma_start(c2_in[:], c2_sb[:])
                collective("AllReduce", OP.add, c2_in[:], c2_out[:])

                # ---- derived scalars from c1 (g,u) ----
                c1g = sbC.tile([1, 8], f32)
                nc.sync.dma_start(c1g[:], c1_out[:])
                m_w = sbC.tile([1, 2], f32)
                nc.vector.tensor_scalar(out=m_w[:], in0=c1g[:, 0:2],
                                        scalar1=1.0 / (float(F) * D),
                                        scalar2=QEPS, op0=OP.mult, op1=OP.max)
                s_w = sbC.tile([1, 2], f32)
                nc.vector.reciprocal(s_w[:], m_w[:])
                bcast_row(m_w_col, m_w, 2)
                bcast_row(s_w_col, s_w, 2)

                # S_bcast = 127 / absmax, broadcast down partitions (PE)
                for j in range(0, T, 512):
                    scr = sbS2.tile([1, 512], f32, tag="scr")
                    nc.sync.dma_start(scr[:], sc_out[0:1, j:j + 512])
                    pb = psH.tile([128, 512], f32, tag="pb", bufs=2)
                    nc.tensor.matmul(pb[:], ones1[:], scr[:],
                                     start=True, stop=True)
                    rw = sbS2.tile([128, 512], f32, tag="rw", bufs=2)
                    nc.vector.reciprocal(rw[:], pb[:])
                    nc.vector.tensor_scalar(out=S_bcast[:, j:j + 512],
                                            in0=rw[:], scalar1=127.0,
                                            scalar2=None, op0=OP.mult)
                yraw = sbC.tile([128, TT], f32)
                nc.sync.dma_start(yraw[:], r128(sc_out[:]))
                nc.vector.tensor_scalar(out=DEQG[:], in0=yraw[:],
                                        scalar1=m_w_col[:, 0:1],
                                        scalar2=1.0 / 127.0,
                                        op0=OP.mult, op1=OP.mult)
                nc.vector.tensor_scalar(out=DEQU[:], in0=yraw[:],
                                        scalar1=m_w_col[:, 1:2],
                                        scalar2=1.0 / 127.0,
                                        op0=OP.mult, op1=OP.mult)

                # second dummy batch bridges the gap to the first real MMs
                if NDUM1 > 0 and NDUM2 > 0:
                    dum2 = psH.tile([128, NF], f32, tag="dum2")
                    for k in range(NDUM2):
                        nc.tensor.matmul(dum2[:], zeros_bf[:, 0:128],
                                         zeros_bf[:], start=(k == 0),
                                         stop=(k == NDUM2 - 1))
                    dsc = sbC.tile([1, 8], f32)
                    nc.vector.memset(dsc[:], 0.0)
                    nc.vector.tensor_copy(dsc[:, 0:1], dum[0:1, 0:1])
                    nc.vector.tensor_copy(dsc[:, 1:2], dum2[0:1, 0:1])
                    nc.sync.dma_start(dum_dram[:], dsc[:])

                # ---- derived scalars from c2 (wd) ----
                c2g = sbC.tile([1, 8], f32)
                nc.sync.dma_start(c2g[:], c2_out[:])
                mws = sbC.tile([1, 2], f32)
                nc.vector.tensor_scalar(out=mws[:, 0:1], in0=c2g[:, 0:1],
                                        scalar1=1.0 / (float(F) * D),
                                        scalar2=QEPS, op0=OP.mult, op1=OP.max)
                nc.vector.reciprocal(mws[:, 1:2], mws[:, 0:1])
                mwsc = sbC.tile([128, 2], f32)
                bcast_row(mwsc, mws, 2)
                nc.vector.tensor_copy(m_wd_col[:], mwsc[:, 0:1])
                nc.vector.tensor_copy(s_wd_col[:], mwsc[:, 1:2])

                if not ln_is_ones:
                    if SAFE_BCAST:
                        lnpad = sbT1.tile([128, FS], f32, tag="T1")
                        nc.vector.memset(lnpad[:], 0.0)
                        nc.sync.dma_start(lnpad[0:1, :], lnw_t.ap())
                        nc.gpsimd.partition_all_reduce(
                            Ln_bcast[:], lnpad[:], channels=128,
                            reduce_op=RO.add)
                    else:
                        lnr = sbC.tile([1, FS], f32)
                        nc.sync.dma_start(lnr[:], lnw_t.ap())
                        nc.gpsimd.partition_broadcast(Ln_bcast[:], lnr[:],
                                                      channels=128)

            # ================= Wg/Wu quantization stream =================
            # per [128, FS] tile: scalar(w*s + MAGIC) -> vector clip ->
            # scalar(-MAGIC, cast bf16). Exact ternary ints in bf16.
            wq_g = [None] * DT
            wq_u = [None] * DT
            for i in range(0, DT, WB):
                for lst, ten, scol in ((wq_g, wgT_t, 0), (wq_u, wuT_t, 1)):
                    wt = sbS8.tile([128, WB * FS], f32, tag="S8")
                    nc.sync.dma_start(
                        wt[:],
                        ten.ap()[i * 128:(i + WB) * 128, :]
                        .rearrange("(b p) c -> p b c", p=128))
                    for b in range(WB):
                        t1 = sbT1.tile([128, FS], f32, tag="T1")
                        nc.scalar.activation(out=t1[:],
                                             in_=wt[:, b * FS:(b + 1) * FS],
                                             func=AF.Copy,
                                             scale=s_w_col[:, scol:scol + 1],
                                             bias=MAGIC)
                        t2 = clip_step(t1)
                        wq = sbWQ.tile([128, FS], bf16, tag="wq")
                        nc.scalar.activation(out=wq[:], in_=t2[:],
                                             func=AF.Copy, bias=-MAGIC)
                        lst[i + b] = wq

            # ================= Wd quantization + S row =================
            def emit_wd():
                for i in range(FS // 128):
                    wt = sbS8.tile([128, D], f32, tag="S8")
                    nc.sync.dma_start(wt[:],
                                      wdT_t.ap()[i * 128:(i + 1) * 128, :])
                    chs = []
                    for b in range(2):
                        t1 = sbT1.tile([128, FS], f32, tag="T1")
                        nc.scalar.activation(out=t1[:],
                                             in_=wt[:, b * FS:(b + 1) * FS],
                                             func=AF.Copy,
                                             scale=s_wd_col[:], bias=MAGIC)
                        t2 = clip_step(t1)
                        wdq = sbT1.tile([128, FS], f32, tag="T1")
                        nc.scalar.activation(out=wdq[:], in_=t2[:],
                                             func=AF.Copy, bias=-MAGIC)
                        ch = sbCol.tile([128, 1], f32, tag=f"wdacc{b}")
                        nc.vector.tensor_reduce(out=ch[:], in_=wdq[:],
                                                axis=AX.X, op=OP.add)
                        chs.append(ch)
                    nc.vector.tensor_tensor(out=Ssh_cols[:, i:i + 1],
                                            in0=chs[0][:], in1=chs[1][:],
                                            op=OP.add)
                # S row -> broadcast down partitions
                nc.sync.dma_start(r128(srow_dram[:]), Ssh_cols[:])
                if SAFE_BCAST:
                    shpad = sbT1.tile([128, FS], f32, tag="T1")
                    nc.vector.memset(shpad[:], 0.0)
                    nc.sync.dma_start(shpad[0:1, :], srow_dram[:])
                    nc.gpsimd.partition_all_reduce(Sh_bcast[:], shpad[:],
                                                   channels=128,
                                                   reduce_op=RO.add)
                else:
                    srow = sbC.tile([1, FS], f32)
                    nc.sync.dma_start(srow[:], srow_dram[:])
                    nc.gpsimd.partition_broadcast(Sh_bcast[:], srow[:],
                                                  channels=128)

            # ================= post-stats (requant h, dot with S) =========
            def emit_post(s):
                SEG = SEGB[s + 1] - SEGB[s]
                t0 = SEGB[s]
                stout = st_out[s]
                ssq_g = sbSG.tile([128, SEG, NCORES], f32, tag="SG")
                am_g = sbSG.tile([128, SEG, NCORES], f32, tag="SG2")
                if SAFE_GATHER:
                    for r in range(NCORES):
                        nc.sync.dma_start(
                            ssq_g[:, :, r:r + 1],
                            stout[256 * r:256 * r + 128, :][:, :, None])
                        nc.sync.dma_start(
                            am_g[:, :, r:r + 1],
                            stout[256 * r + 128:256 * r + 256, :][:, :, None])
                else:
                    view = stout[:].rearrange("(r sp) i -> sp i r", sp=256)
                    nc.sync.dma_start(ssq_g[:], view[0:128])
                    nc.sync.dma_start(am_g[:], view[128:256])
                ssq12 = sbCol.tile([128, SEG], f32, tag="st_a")
                nc.vector.tensor_reduce(out=ssq12[:], in_=ssq_g[:], axis=AX.X,
                                        op=OP.add)
                am12 = sbCol.tile([128, SEG], f32, tag="st_b")
                nc.vector.tensor_reduce(out=am12[:], in_=am_g[:], axis=AX.X,
                                        op=OP.max)
                v = sbCol.tile([128, SEG], f32, tag="st_c")
                nc.vector.tensor_scalar(out=v[:], in0=ssq12[:],
                                        scalar1=1.0 / F, scalar2=EPS,
                                        op0=OP.mult, op1=OP.add)
                sv = sbCol.tile([128, SEG], f32, tag="st_d")
                nc.scalar.activation(out=sv[:], in_=v[:], func=AF.Sqrt)
                rs = sbCol.tile([128, SEG], f32, tag="st_e")
                nc.vector.reciprocal(rs[:], sv[:])
                rg = sbCol.tile([128, SEG], f32, tag="st_f")
                nc.vector.tensor_tensor(out=rg[:], in0=rs[:], in1=am12[:],
                                        op=OP.mult)
                y2 = sbCol.tile([128, SEG], f32, tag="st_g")
                nc.vector.tensor_scalar(out=y2[:], in0=rg[:], scalar1=QEPS,
                                        scalar2=None, op0=OP.max)
                invs2 = sbCol.tile([128, SEG], f32, tag="st_h")
                nc.vector.tensor_scalar(
                    out=invs2[:], in0=y2[:], scalar1=m_wd_col[:],
                    scalar2=1.0 / (127.0 * float(H) * D),
                    op0=OP.mult, op1=OP.mult)
                r2 = sbCol.tile([128, SEG], f32, tag="st_i")
                nc.vector.reciprocal(r2[:], y2[:])
                alpha = sbCol.tile([128, SEG], f32, tag="st_j")
                nc.vector.tensor_tensor(out=alpha[:], in0=r2[:], in1=rs[:],
                                        op=OP.mult)
                alpha2 = sbCol.tile([128, SEG], f32, tag="st_k")
                nc.vector.tensor_scalar(out=alpha2[:], in0=alpha[:],
                                        scalar1=127.0, scalar2=None,
                                        op0=OP.mult)
                for i in range(SEG):
                    t = t0 + i
                    w1 = sbT1.tile([128, FS], f32, tag="T1")
                    nc.vector.tensor_scalar(out=w1[:], in0=ht_tiles[t][:],
                                            scalar1=alpha2[:, i:i + 1],
                                            scalar2=MAGIC, op0=OP.mult,
                                            op1=OP.add)
                    hq = sbT1.tile([128, FS], f32, tag="T1")
                    nc.scalar.activation(out=hq[:], in_=w1[:],
                                         func=AF.Identity, bias=negmagic[:])
                    qacc = sbCol.tile([128, 1], f32, tag="qacc")
                    mult_rowsum(hq[:], Sh_bcast[:], qacc, None)
                    nc.vector.tensor_scalar(out=Q_cols[:, t:t + 1],
                                            in0=qacc[:],
                                            scalar1=invs2[:, i:i + 1],
                                            scalar2=None, op0=OP.mult)

            # ================= main matmul loop =================
            def emit_xq(gi, xq_slabs):
                t0, gsz = GROUPS[gi]
                gw = gsz * 128
                tc0 = t0 * 128
                for d0 in range(0, DT, WB):
                    xsl = sbXQ.tile([128, WB * 512], f32, tag="xsl", bufs=2)
                    nc.sync.dma_start(
                        xsl[:, 0:WB * gw],
                        xT_t.ap()[d0 * 128:(d0 + WB) * 128, tc0:tc0 + gw]
                        .rearrange("(b p) c -> p b c", p=128))
                    for b in range(WB):
                        xp = sbXQ.tile([128, 512], f32, tag="xp", bufs=2)
                        nc.vector.tensor_tensor(
                            out=xp[:, 0:gw], in0=xsl[:, b * gw:(b + 1) * gw],
                            in1=S_bcast[:, tc0:tc0 + gw], op=OP.mult)
                        xq = sbXQ.tile([128, 512], bf16, tag="xq")
                        nc.vector.tensor_scalar(out=xq[:, 0:gw],
                                                in0=xp[:, 0:gw],
                                                scalar1=MAGIC, scalar2=MAGIC,
                                                op0=OP.add, op1=OP.subtract)
                        xq_slabs[d0 + b] = xq
                return xq_slabs

            emit_wd()

            with tc.tile_pool(name="psM", bufs=2, space="PSUM") as psM:
                cur_xq = emit_xq(0, [None] * DT)
                nxt_xq = None
                for gi, (t0, gsz) in enumerate(GROUPS):
                    for s in POST_AT.get(gi, []):
                        emit_post(s)
                    for tl in range(gsz):
                        t = t0 + tl
                        tc0 = tl * 128
                        gps = [psM.tile([128, NF], f32, tag=f"g{j}",
                                        name=f"gp{j}") for j in range(FH)]
                        ups = [psM.tile([128, NF], f32, tag=f"u{j}",
                                        name=f"up{j}") for j in range(FH)]
                        for d in range(DT):
                            lhsT = cur_xq[d][:, tc0:tc0 + 128]
                            s0, s1 = (d == 0), (d == DT - 1)
                            for j in range(FH):
                                nc.tensor.matmul(gps[j][:], lhsT,
                                                 wq_g[d][:, j * NF:(j + 1) * NF],
                                                 start=s0, stop=s1)
                                nc.tensor.matmul(ups[j][:], lhsT,
                                                 wq_u[d][:, j * NF:(j + 1) * NF],
                                                 start=s0, stop=s1)
                        us = sbUG.tile([128, FS], fp16, tag="us")
                        gsl = sbUG.tile([128, FS], fp16, tag="gs")
                        for j in range(FH):
                            nc.scalar.activation(out=us[:, j * NF:(j + 1) * NF],
                                                 in_=ups[j][:], func=AF.Copy,
                                                 scale=DEQU[:, t:t + 1])
                            nc.scalar.activation(out=gsl[:, j * NF:(j + 1) * NF],
                                                 in_=gps[j][:], func=AF.Silu,
                                                 scale=DEQG[:, t:t + 1])
                        ht = sbH.tile([128, FS], fp16, tag="h")
                        ht_tiles[t] = ht
                        if ln_is_ones:
                            nc.vector.tensor_tensor(out=ht[:], in0=gsl[:],
                                                    in1=us[:], op=OP.mult)
                            hsq = sbUG.tile([128, FS], fp16, tag="hsq", bufs=1)
                            nc.scalar.activation(
                                out=hsq[:], in_=ht[:], func=AF.Square,
                                accum_out=ssq_cols[:, t:t + 1])
                            nc.vector.tensor_reduce(
                                out=am_cols[:, t:t + 1], in_=ht[:], axis=AX.X,
                                op=OP.max, apply_absolute_value=True)
                        else:
                            htf = sbT1.tile([128, FS], f32, tag="T1")
                            nc.vector.tensor_tensor(out=htf[:], in0=gsl[:],
                                                    in1=us[:], op=OP.mult)
                            hsq = sbUG.tile([128, FS], fp16, tag="hsq", bufs=1)
                            nc.scalar.activation(
                                out=hsq[:], in_=htf[:], func=AF.Square,
                                accum_out=ssq_cols[:, t:t + 1])
                            nc.vector.tensor_tensor(out=ht[:], in0=htf[:],
                                                    in1=Ln_bcast[:],
                                                    op=OP.mult)
                            nc.vector.tensor_reduce(
                                out=am_cols[:, t:t + 1], in_=ht[:], axis=AX.X,
                                op=OP.max, apply_absolute_value=True)
                        # segment boundary: ship stats, trigger AllGather
                        for s in range(NSEG):
                            if t == SEGB[s + 1] - 1:
                                a, b2 = SEGB[s], SEGB[s + 1]
                                nc.sync.dma_start(st_in[s][0:128, :],
                                                  ssq_cols[:, a:b2])
                                nc.sync.dma_start(st_in[s][128:256, :],
                                                  am_cols[:, a:b2])
                                collective("AllGather", OP.bypass,
                                           st_in[s][:], st_out[s][:])
                        # prefetch next group's xq after the 2nd tile
                        if tl == min(1, gsz - 1) and gi + 1 < len(GROUPS):
                            nxt_xq = emit_xq(gi + 1, [None] * DT)
                    if gi + 1 < len(GROUPS):
                        cur_xq, nxt_xq = nxt_xq, None

            for s in POST_TAIL:
                emit_post(s)

            # ============ pooled partials + classifier ============
            with tc.tile_pool(name="psE", bufs=1, space="PSUM") as psE:
                pq = psE.tile([1, TT], f32, tag="pq")
                nc.tensor.matmul(pq[:], ones_col[:], Q_cols[:],
                                 start=True, stop=True)
                plrow = sbC.tile([1, TT], f32)
                nc.vector.tensor_copy(plrow[:], pq[:])
                nc.sync.dma_start(pl_in[:], plrow[:])
                collective("AllReduce", OP.add, pl_in[:], pl_out[:])

                pool3 = sbC.tile([C, B], f32)
                nc.sync.dma_start(
                    pool3[:], pl_out[:].rearrange("o (b c) -> (o c) b", c=C))
                clsW_sb = sbC.tile([C, NCLS], f32)
                nc.sync.dma_start(clsW_sb[:], clsWT_t.ap())
                clsb_sb = sbC.tile([1, NCLS], f32)
                nc.sync.dma_start(clsb_sb[:], clsb_t.ap())
                out_sb = sbC.tile([B, NCLS], f32)
                for j in range(0, NCLS, 512):
                    w = min(512, NCLS - j)
                    pcls = psE.tile([B, 512], f32, tag="pcls", bufs=2)
                    nc.tensor.matmul(pcls[:, 0:w], pool3[:],
                                     clsW_sb[:, j:j + w], start=True,
                                     stop=False)
                    nc.tensor.matmul(pcls[:, 0:w], ones1[:, 0:B],
                                     clsb_sb[:, j:j + w], start=False,
                                     stop=True)
                    nc.vector.tensor_copy(out_sb[:, j:j + w], pcls[:, 0:w])
                nc.sync.dma_start(out_t.ap(), out_sb[:])

    nc.compile()
    meta = dict(B=B, C=C, H=H, D=D, F=F, NCLS=NCLS, NCORES=NCORES,
                T=T, TS=TS, FS=FS)
    return nc, meta


def make_in_maps(x, Wg, Wu, Wd, ln_w, cls_W, cls_b, meta):
    """Host-side sharding: slices/transposes only, no arithmetic."""
    T, TS, FS = meta["T"], meta["TS"], meta["FS"]
    D = meta["D"]
    NCLS = meta["NCLS"]
    NCORES = meta["NCORES"]
    xf = np.ascontiguousarray(np.asarray(x, np.float32).reshape(T, D))
    xT = np.ascontiguousarray(xf.T)
    clsWT = np.ascontiguousarray(np.asarray(cls_W, np.float32).T)
    clsb2 = np.ascontiguousarray(np.asarray(cls_b, np.float32).reshape(1, NCLS))
    maps = []
    for k in range(NCORES):
        f0 = k * FS
        maps.append({
            "xs": np.ascontiguousarray(xf[k * TS:(k + 1) * TS]),
            "xT": xT,
            "wgT": np.ascontiguousarray(np.asarray(Wg, np.float32)[f0:f0 + FS, :].T),
            "wuT": np.ascontiguousarray(np.asarray(Wu, np.float32)[f0:f0 + FS, :].T),
            "wdT": np.ascontiguousarray(np.asarray(Wd, np.float32)[:, f0:f0 + FS].T),
            "lnw": np.ascontiguousarray(np.asarray(ln_w, np.float32)[f0:f0 + FS].reshape(1, FS)),
            "clsWT": clsWT,
            "clsb": clsb2,
        })
    return maps


_CACHE = {}


def kernel(x, Wg, Wu, Wd, ln_w, cls_W, cls_b):
    """Takes FULL inputs, runs the 8-core SPMD Bass kernel, returns [B, NCLS]."""
    from concourse import bass_utils

    x = np.asarray(x, np.float32)
    B, C, H, D = x.shape
    F = int(np.asarray(Wg).shape[0])
    NCLS = int(np.asarray(cls_W).shape[0])
    ln_ones = bool(np.all(np.asarray(ln_w) == 1.0))
    key = (B, C, H, D, F, NCLS, ln_ones)
    if key not in _CACHE:
        _CACHE[key] = build(B=B, C=C, H=H, D=D, F=F, NCLS=NCLS, NCORES=8,
                            ln_is_ones=ln_ones)
    nc, meta = _CACHE[key]
    in_maps = make_in_maps(x, Wg, Wu, Wd, ln_w, cls_W, cls_b, meta)
    res = bass_utils.run_bass_kernel_spmd(nc, in_maps, core_ids=list(range(8)))
    return np.asarray(res.results[0]["out"], np.float32)
